# revision 3
# baseline (speedup 1.0000x reference)
"""ConceptContrastiveLoss Trainium2 kernel v2 (8-core SPMD, batch-parallel,
overlapped centroid AllGathers).

Differences vs v1 baseline (200us graded):
- Bulk processes expert batches first (groups 0..3), then violator
  (groups 4..7).  As soon as the expert centroids are finalized, a
  [D, Bl] AllGather is issued on the gpsimd queue -- it completes while
  the violator half of the bulk DMA phase (~80us) is still running, so
  its collective latency is fully hidden.
- The expert-only pairwise work (norms matmuls, EE clustering term) is
  issued AFTER all bulk instructions on each engine queue, so it blocks
  nothing and executes in the shadow of the second (violator) gather.
- Only the violator-dependent work (EV separation, VV clustering,
  combine, out DMA) remains after the second gather completes.
- Both loc-centroid DMAs, both gathers and both SBUF reloads live on the
  gpsimd queue (SWDGE), which carries no bulk work, so their semaphore
  waits never stall the bulk DMA queues (sync + scalar HWDGE).

Build knobs (for measurement):
- overlap=False: single end-of-bulk AllGather of all 64 centroid
  columns + single PE gate (v1-equivalent tail structure).
- full_loop=True: wrap the ENTIRE kernel (bulk + gathers + pairwise +
  out DMA) in For_i(0, loop_r) -- per-iteration loop-delta then measures
  the complete kernel, which is what the harness grades.
- probe="dma": bulk DMAs only (no compute, no gathers) -- HBM floor.
"""

from contextlib import ExitStack, nullcontext

import numpy as np

import concourse.bacc as bacc
import concourse.bass as bass
import concourse.mybir as mybir
import concourse.tile as tile
from concourse.bass_utils import run_bass_kernel_spmd
from concourse.tile import add_dep_helper

F32 = mybir.dt.float32

MARGIN = 10.0
ALPHA = 3.0
BETA = 0.3
GAMMA = 0.3

B, S, D = 256, 2048, 128
N_CORES = 8
BPD = 1   # batch items per DMA (DMA size = BPD MiB)
BUFS = 8  # big-tile pool buffers
DVE_STOP = 128       # halving-tree handoff width (elems); PE folds the rest
N_DMA_ENG = 2        # DMA issue streams: 2 = SP+ACT HWDGE


def _build_body(tc, e, v, out, loc_cE, loc_cV, gathE, gathV, B, S, D, n_cores,
                solo=False, bpd=BPD, bufs=BUFS, loop_r=1, dve_stop=DVE_STOP,
                n_dma_eng=N_DMA_ENG, overlap=True, full_loop=False, probe=None):
    nc = tc.nc
    Bl = B // n_cores  # local batches per tensor
    J = S // 128       # seq tiles per batch item
    n_pairs = B * (B - 1) // 2
    w_ev = ALPHA / (B * B)
    w_ee = BETA / (2.0 * n_pairs)
    w_vv = GAMMA / (2.0 * n_pairs)
    blks = [(s, min(128, B - s)) for s in range(0, B, 128)]
    n_blk = len(blks)

    with ExitStack() as ctx:
        # ---- pools (all hoisted outside any hardware loop) ----
        consts = ctx.enter_context(tc.tile_pool(name="consts", bufs=1))
        big_pool = ctx.enter_context(tc.tile_pool(name="big", bufs=bufs))
        cps = ctx.enter_context(tc.tile_pool(name="cps", bufs=4, space="PSUM"))
        sp = ctx.enter_context(tc.tile_pool(name="sp", bufs=1))
        sps = ctx.enter_context(tc.tile_pool(name="sps", bufs=1, space="PSUM"))
        spp = ctx.enter_context(tc.tile_pool(name="spp", bufs=2, space="PSUM"))
        spf = ctx.enter_context(tc.tile_pool(name="spf", bufs=1, space="PSUM"))
        trash_pool = ctx.enter_context(tc.tile_pool(name="trash", bufs=2))

        ones_col = nc.const_aps.aps[(F32, 1.0)]
        b_eps = consts.tile([128, 1], F32, name="b_eps")
        nc.vector.memset(b_eps[:], 1e-12)
        b_margin = consts.tile([128, 1], F32, name="b_margin")
        nc.vector.memset(b_margin[:], MARGIN)
        centS = consts.tile([D, 2 * Bl], F32, name="centS")
        fin = sp.tile([1, 1], F32, name="fin")

        cent_copies_E = []
        cent_copies_V = []
        GRP = min(8, Bl)
        n_groups_per_t = (Bl + GRP - 1) // GRP
        assert GRP % bpd == 0 and Bl % bpd == 0
        dma_engines = [nc.sync, nc.scalar, nc.gpsimd][:n_dma_eng]
        dma_i = [0]

        def emit_bulk_half(t_idx):
            # t_idx 0 = expert (centS cols 0:Bl), 1 = violator (Bl:2Bl)
            src = (e, v)[t_idx]
            copies = (cent_copies_E, cent_copies_V)[t_idx]
            for gl in range(n_groups_per_t):
                G = None
                if probe != "dma":
                    G = cps.tile([128, 512], F32, name="Gacc")
                start_mm = None
                col_last = []
                for ti in range(GRP // bpd):
                    b0 = gl * GRP + ti * bpd
                    Tb = big_pool.tile([128, bpd * J * D], F32, name="Tb")
                    eng = dma_engines[dma_i[0] % len(dma_engines)]
                    dma_i[0] += 1
                    if bpd == 1:
                        eng.dma_start(
                            out=Tb[:],
                            in_=src[b0].rearrange("(p j) d -> p (j d)", p=128),
                        )
                    else:
                        eng.dma_start(
                            out=Tb[:].rearrange("p (b x) -> p b x", b=bpd),
                            in_=src[b0 : b0 + bpd].rearrange(
                                "b (p j) d -> p b (j d)", p=128
                            ),
                        )
                    if probe == "dma":
                        continue
                    # tree-halve the 16 seq rows per partition on DVE down to
                    # width dve_stop; PE folds the rest into one PSUM column
                    w = J * D // 2
                    Tb3 = (
                        Tb[:].rearrange("p (b x) -> p b x", b=bpd)
                        if bpd > 1
                        else None
                    )
                    while w >= dve_stop:
                        if bpd == 1:
                            nc.vector.tensor_add(
                                Tb[:, 0:w], Tb[:, 0:w], Tb[:, w : 2 * w]
                            )
                        else:
                            nc.vector.tensor_add(
                                Tb3[:, :, 0:w], Tb3[:, :, 0:w],
                                Tb3[:, :, w : 2 * w],
                            )
                        w //= 2
                    for bi in range(bpd):
                        c = ti * bpd + bi
                        base = bi * J * D
                        n_folds = dve_stop // D
                        for fi in range(n_folds):
                            is_first = c == 0 and fi == 0
                            is_last = c == GRP - 1 and fi == n_folds - 1
                            o = base + fi * D
                            mm = nc.tensor.matmul(
                                out=G[:, c : c + 1],
                                lhsT=Tb[:, o : o + D],
                                rhs=ones_col,
                                start=is_first,
                                stop=is_last,
                            )
                            if start_mm is None:
                                start_mm = mm
                            elif fi == 0:
                                add_dep_helper(
                                    mm.ins, start_mm.ins, sync=False,
                                    reason="psum group start first",
                                )
                            if fi == n_folds - 1:
                                col_last.append(mm)
                            if is_last:
                                for prev in col_last[:-1]:
                                    add_dep_helper(
                                        mm.ins, prev.ins, sync=False,
                                        reason="psum group stop last",
                                    )
                if probe != "dma":
                    c0 = t_idx * Bl + gl * GRP
                    copies.append(
                        nc.scalar.mul(
                            centS[:, c0 : c0 + GRP], G[:, 0:GRP], 1.0 / S
                        )
                    )

        def gather_chain(loc, gath, cols, reload_out, reload_in):
            nc.gpsimd.dma_start(out=loc[:], in_=centS[:, cols[0] : cols[1]])
            if solo:
                # timing-only stand-in for the collective: one local copy
                # with the same dependency structure (output values invalid)
                nc.gpsimd.dma_start(out=gath[0:D, :], in_=loc[:])
            else:
                nc.gpsimd.collective_compute(
                    "AllGather",
                    mybir.AluOpType.bypass,
                    replica_groups=[list(range(n_cores))],
                    ins=[loc[:]],
                    outs=[gath[:]],
                )
            return nc.gpsimd.dma_start(out=reload_out, in_=reload_in)

        def pe_mm(gate, *args, **kwargs):
            mm = nc.tensor.matmul(*args, **kwargs)
            add_dep_helper(mm.ins, gate.ins, sync=False, reason="after gate")
            return mm

        if overlap:
            CtE = sp.tile([D, B], F32, name="CtE")
            CtV = sp.tile([D, B], F32, name="CtV")
        else:
            CtEV = sp.tile([D, 2 * B], F32, name="CtEV")
            CtE = CtEV[:, 0:B]
            CtV = CtEV[:, B : 2 * B]
        m2E = sp.tile([D, B], F32, name="m2E")
        m2V = sp.tile([D, B], F32, name="m2V")
        sqE = sp.tile([D, B], F32, name="sqE")
        sqV = sp.tile([D, B], F32, name="sqV")
        acc = sp.tile([128, 3 * n_blk], F32, name="acc")
        # aug tiles: rows 0 and 32 carry {norms, ones}; rest zero (K=64)
        ag_e = sp.tile([64, B], F32, name="ag_e")    # lhsT rows: n_e, 1
        ag_v = sp.tile([64, B], F32, name="ag_v")    # lhsT rows: n_v, 1
        rhs_e = sp.tile([64, B], F32, name="rhs_e")  # rhs rows: 1, n_e
        rhs_v = sp.tile([64, B], F32, name="rhs_v")  # rhs rows: 1, n_v
        t_ee = sp.tile([128, 1], F32, name="t_ee")
        t_ev = sp.tile([128, 1], F32, name="t_ev")
        t_vv = sp.tile([128, 1], F32, name="t_vv")
        tot = sp.tile([128, 1], F32, name="tot")

        loop_cm = tc.For_i(0, loop_r, 1) if full_loop else nullcontext()
        with loop_cm:
            if full_loop or loop_r == 1:
                emit_bulk_half(0)
                emit_bulk_half(1)
            else:
                with tc.For_i(0, loop_r, 1) as _i:
                    emit_bulk_half(0)
                    emit_bulk_half(1)

            if probe == "dma":
                nc.vector.memset(fin[:], 0.0)
                nc.sync.dma_start(out=out[:], in_=fin[:])
                return

            # ---- centroid exchange on the gpsimd queue ----
            if overlap:
                dmaE = gather_chain(
                    loc_cE, gathE, (0, Bl),
                    CtE[:].rearrange("p (c j) -> p c j", c=n_cores),
                    gathE.rearrange("(c p) j -> p c j", c=n_cores),
                )
                dmaV = gather_chain(
                    loc_cV, gathV, (Bl, 2 * Bl),
                    CtV[:].rearrange("p (c j) -> p c j", c=n_cores),
                    gathV.rearrange("(c p) j -> p c j", c=n_cores),
                )
            else:
                dmaE = dmaV = gather_chain(
                    loc_cE, gathE, (0, 2 * Bl),
                    CtEV[:].rearrange("p (t c j) -> p t c j", t=2, c=n_cores),
                    gathE.rearrange("(c p) (t j) -> p t c j", c=n_cores, t=2),
                )

            # ---- pairwise phase ----
            nc.vector.memset(acc[:], 0.0)
            psn = sps.tile([128, 512], F32, name="psn", tag="psn")

            # --- E-only part (hidden under the V gather when overlap) ---
            gateE = nc.tensor.nop()
            add_dep_helper(gateE.ins, dmaE.ins, sync=True, reason="gate ctE")
            nc.vector.tensor_scalar_mul(m2E[:], CtE, -2.0)
            nc.vector.tensor_mul(sqE[:], CtE, CtE)
            # squared E norms at partition 0 (ag_e) and partition 32 (rhs_e)
            pe_mm(gateE, out=psn[0:1, 0:B], lhsT=ones_col, rhs=sqE[:])
            pe_mm(gateE, out=psn[32:33, B : 2 * B], lhsT=ones_col, rhs=sqE[:])
            for t, row0_src, row32_src in (
                (ag_e, psn[0:1, 0:B], None),
                (rhs_e, None, psn[32:33, B : 2 * B]),
            ):
                nc.vector.memset(t[:], 0.0)
                if row0_src is not None:
                    nc.vector.tensor_copy(t[0:1, :], row0_src)
                    nc.vector.memset(t[32:33, :], 1.0)
                else:
                    nc.vector.memset(t[0:1, :], 1.0)
                    nc.vector.tensor_copy(t[32:33, :], row32_src)

            # EE clustering: sq = -2 E^T E + (n_i + n_j), relu (acc cols 0:2)
            for bi, (bs, bn) in enumerate(blks):
                P_clb = spp.tile([128, 512], F32, name="P_cl", tag="P_cl")
                P_cl = P_clb[:, 0:B]
                pe_mm(gateE, out=P_cl[:bn], lhsT=m2E[:, bs : bs + bn],
                      rhs=CtE, start=True, stop=False)
                pe_mm(gateE, out=P_cl[:bn], lhsT=ag_e[:, bs : bs + bn],
                      rhs=rhs_e[:], start=False, stop=True)
                rel = trash_pool.tile([128, B], F32, name="rel", tag="rel")
                nc.scalar.activation(
                    rel[:bn], P_cl[:bn], mybir.ActivationFunctionType.Relu,
                    accum_out=acc[:bn, bi : bi + 1],
                )

            # --- V-dependent part (exposed tail) ---
            gateV = nc.tensor.nop()
            add_dep_helper(gateV.ins, dmaV.ins, sync=True, reason="gate ctV")
            nc.vector.tensor_scalar_mul(m2V[:], CtV, -2.0)
            nc.vector.tensor_mul(sqV[:], CtV, CtV)
            # squared V norms at partition 0 (ag_v) and partition 32 (rhs_v)
            pe_mm(gateV, out=psn[0:1, B : 2 * B], lhsT=ones_col, rhs=sqV[:])
            pe_mm(gateV, out=psn[32:33, 0:B], lhsT=ones_col, rhs=sqV[:])
            for t, row0_src, row32_src in (
                (ag_v, psn[0:1, B : 2 * B], None),
                (rhs_v, None, psn[32:33, 0:B]),
            ):
                nc.vector.memset(t[:], 0.0)
                if row0_src is not None:
                    nc.vector.tensor_copy(t[0:1, :], row0_src)
                    nc.vector.memset(t[32:33, :], 1.0)
                else:
                    nc.vector.memset(t[0:1, :], 1.0)
                    nc.vector.tensor_copy(t[32:33, :], row32_src)

            # EV separation: sq = -2 E^T V + (n_e + n_v), hinge (acc 2:4)
            for bi, (bs, bn) in enumerate(blks):
                P_evb = spp.tile([128, 512], F32, name="P_ev", tag="P_cl")
                P_ev = P_evb[:, 0:B]
                pe_mm(gateV, out=P_ev[:bn], lhsT=m2E[:, bs : bs + bn],
                      rhs=CtV, start=True, stop=False)
                pe_mm(gateV, out=P_ev[:bn], lhsT=ag_e[:, bs : bs + bn],
                      rhs=rhs_v[:], start=False, stop=True)
                dist = trash_pool.tile([128, B], F32, name="dist", tag="rel")
                hin = trash_pool.tile([128, B], F32, name="hin", tag="rel")
                hsq = trash_pool.tile([128, B], F32, name="hsq", tag="rel")
                nc.vector.tensor_scalar_max(P_ev[:bn], P_ev[:bn], 0.0)
                nc.scalar.activation(
                    dist[:bn], P_ev[:bn], mybir.ActivationFunctionType.Sqrt,
                    bias=b_eps[:bn],
                )
                nc.scalar.activation(
                    hin[:bn], dist[:bn], mybir.ActivationFunctionType.Relu,
                    bias=b_margin[:bn], scale=-1.0,
                )
                nc.scalar.activation(
                    hsq[:bn], hin[:bn], mybir.ActivationFunctionType.Square,
                    accum_out=acc[:bn, n_blk + bi : n_blk + bi + 1],
                )

            # VV clustering: sq = -2 V^T V + (n_i + n_j), relu (acc 4:6)
            for bi, (bs, bn) in enumerate(blks):
                P_clb = spp.tile([128, 512], F32, name="P_vv", tag="P_cl")
                P_cl = P_clb[:, 0:B]
                pe_mm(gateV, out=P_cl[:bn], lhsT=m2V[:, bs : bs + bn],
                      rhs=CtV, start=True, stop=False)
                pe_mm(gateV, out=P_cl[:bn], lhsT=ag_v[:, bs : bs + bn],
                      rhs=rhs_v[:], start=False, stop=True)
                rel = trash_pool.tile([128, B], F32, name="rel2", tag="rel")
                ci = 2 * n_blk + bi
                nc.scalar.activation(
                    rel[:bn], P_cl[:bn], mybir.ActivationFunctionType.Relu,
                    accum_out=acc[:bn, ci : ci + 1],
                )

            # combine: tot = w_ee*sum(ee) + w_ev*sum(ev) + w_vv*sum(vv)
            for t, base in ((t_ee, 0), (t_ev, n_blk), (t_vv, 2 * n_blk)):
                if n_blk == 1:
                    nc.vector.tensor_copy(t[:], acc[:, base : base + 1])
                else:
                    nc.vector.tensor_add(
                        t[:], acc[:, base : base + 1],
                        acc[:, base + 1 : base + 2],
                    )
                    for k in range(2, n_blk):
                        nc.vector.tensor_add(
                            t[:], t[:], acc[:, base + k : base + k + 1]
                        )
            nc.vector.tensor_scalar_mul(tot[:], t_ev[:], w_ev)
            nc.vector.scalar_tensor_tensor(
                tot[:], t_ee[:], w_ee, tot[:],
                op0=mybir.AluOpType.mult, op1=mybir.AluOpType.add,
            )
            nc.vector.scalar_tensor_tensor(
                tot[:], t_vv[:], w_vv, tot[:],
                op0=mybir.AluOpType.mult, op1=mybir.AluOpType.add,
            )
            psFb = spf.tile([128, 512], F32, name="psF", tag="psF")
            psF = psFb[0:1, 0:1]
            pe_mm(gateV, out=psF, lhsT=ones_col, rhs=tot[:])
            nc.scalar.copy(fin[:], psF)
            nc.sync.dma_start(out=out[:], in_=fin[:])


def build_nc(B=B, S=S, D=D, n_cores=N_CORES, solo=False, bpd=None, bufs=None,
             loop_r=1, dve_stop=None, n_dma_eng=None, overlap=True,
             full_loop=False, probe=None):
    Bl = B // n_cores
    nc = bacc.Bacc("TRN2", num_devices=n_cores)
    e = nc.dram_tensor("expert_concepts", [Bl, S, D], F32, kind="ExternalInput").ap()
    v = nc.dram_tensor("violator_concepts", [Bl, S, D], F32, kind="ExternalInput").ap()
    out = nc.dram_tensor("out", [1, 1], F32, kind="ExternalOutput").ap()
    gath_space = "Local" if solo else "Shared"
    if overlap:
        loc_cE = nc.dram_tensor("loc_cE", [D, Bl], F32).ap()
        loc_cV = nc.dram_tensor("loc_cV", [D, Bl], F32).ap()
        gathE = nc.dram_tensor(
            "gathE", [n_cores * D, Bl], F32, addr_space=gath_space
        ).ap()
        gathV = nc.dram_tensor(
            "gathV", [n_cores * D, Bl], F32, addr_space=gath_space
        ).ap()
    else:
        loc_cE = nc.dram_tensor("loc_cEV", [D, 2 * Bl], F32).ap()
        loc_cV = None
        gathE = nc.dram_tensor(
            "gathEV", [n_cores * D, 2 * Bl], F32, addr_space=gath_space
        ).ap()
        gathV = None
    with tile.TileContext(nc) as tc:
        _build_body(
            tc, e, v, out, loc_cE, loc_cV, gathE, gathV, B, S, D, n_cores,
            solo=solo,
            bpd=bpd if bpd is not None else BPD,
            bufs=bufs if bufs is not None else BUFS,
            loop_r=loop_r,
            dve_stop=dve_stop if dve_stop is not None else DVE_STOP,
            n_dma_eng=n_dma_eng if n_dma_eng is not None else N_DMA_ENG,
            overlap=overlap,
            full_loop=full_loop,
            probe=probe,
        )
    nc.compile()
    return nc


def _run(expert_concepts, violator_concepts, **spmd_kwargs):
    expert_concepts = np.ascontiguousarray(expert_concepts, dtype=np.float32)
    violator_concepts = np.ascontiguousarray(violator_concepts, dtype=np.float32)
    assert expert_concepts.shape == (B, S, D)
    assert violator_concepts.shape == (B, S, D)

    nc = build_nc()
    Bl = B // N_CORES
    in_maps = [
        {
            "expert_concepts": expert_concepts[c * Bl : (c + 1) * Bl],
            "violator_concepts": violator_concepts[c * Bl : (c + 1) * Bl],
        }
        for c in range(N_CORES)
    ]
    res = run_bass_kernel_spmd(nc, in_maps, list(range(N_CORES)), **spmd_kwargs)
    return np.float32(res.results[0]["out"][0, 0]), res


def kernel(expert_concepts: np.ndarray, violator_concepts: np.ndarray) -> np.ndarray:
    out, _ = _run(expert_concepts, violator_concepts)
    return out
